# revision 10
# baseline (speedup 1.0000x reference)
"""Trainium2 Bass kernel for ContinuousConvolutionBase (identity-kernel unfold).

reference:
    delta_times[b, k, t]        = times[b, t] - (times[b, t - s] if t >= s else 0),  s = K-1-k
    pre_conv_features[b, k, t]  = features[b, t - s, :] if t >= s else 0

Sharding: pure data parallel over the batch dim, 2 batches per core on 8 cores.

Per-core kernel strategy (memory-bound; output is ~9x input):
  - features[b] (4096x128 f32 = 2 MB) is loaded once into SBUF as a flat
    [128, 4096] tile (16 KB contiguous per partition -> near-line-rate DMA).
  - Each shifted output plane pre[b,k] is a flat memcpy of that tile into DRAM
    at a float offset of s*C, done as: tiny zero head + [127, 4096] main DMA +
    tail row.  9 shifts x 2 batches = 18 big stores (~2 MB each).
  - delta_times is an 18x4096 vector subtract of two DMA-built tiles
    (broadcast rows minus shifted rows), one 1.2 MB store.
  Loads are issued on the scalar-engine HWDGE ring, stores on the sync ring,
  so load(b=1) overlaps the stores of b=0.
"""

import numpy as np

N_CORES = 8
B_FULL, S, C, K = 16, 4096, 128, 9
BPC = B_FULL // N_CORES  # batches per core
FPP = S * C // 128       # floats per partition of the flat feature tile (4096)

_NC = None


def _build_nc():
    import concourse.mybir as mybir
    from concourse.bacc import Bacc
    from concourse.tile import TileContext

    F32 = mybir.dt.float32
    # Bacc (not plain Bass): its finalize() runs generate_event_semaphores,
    # which splits multi-waits (HW allows 1 sync wait per instruction).
    nc = Bacc()

    times = nc.dram_tensor("times", [BPC, S], F32, kind="ExternalInput")
    feats = nc.dram_tensor("features", [BPC, S, C], F32, kind="ExternalInput")
    delta = nc.dram_tensor("delta", [BPC, K, S], F32, kind="ExternalOutput")
    pre = nc.dram_tensor("pre", [BPC, K, S, C], F32, kind="ExternalOutput")

    with TileContext(nc) as tc:
        with (
            tc.tile_pool(name="small", bufs=1) as small,
            tc.tile_pool(name="fpool", bufs=2) as fpool,
        ):
            # ---------------- delta_times path (small) ----------------
            tb = small.tile([BPC * K, S], F32, name="tb")  # broadcast rows
            sh = small.tile([BPC * K, S], F32, name="sh")  # shifted rows
            dd = small.tile([BPC * K, S], F32, name="dd")
            # zero the t < s heads for every row at once (compute ops must start
            # at partition 0); the shifted DMAs below overwrite cols [s, K-1)
            # per row — Tile's WAW tracking orders them after this memset.
            nc.vector.memset(sh[:, 0 : K - 1], 0.0)
            for b in range(BPC):
                for k in range(K):
                    s = K - 1 - k
                    r = b * K + k
                    nc.scalar.dma_start(out=tb[r : r + 1, :], in_=times[b : b + 1, :])
                    nc.scalar.dma_start(
                        out=sh[r : r + 1, s:S], in_=times[b : b + 1, 0 : S - s]
                    )
            nc.vector.tensor_sub(dd[:, :], tb[:, :], sh[:, :])
            nc.sync.dma_start(out=delta[:].rearrange("b k s -> (b k) s"), in_=dd[:, :])

            # zero source for the t < s head of each output plane (max 1024 floats)
            zt = small.tile([1, (K - 1) * C], F32, name="zt")
            nc.vector.memset(zt[:, :], 0.0)

            # ---------------- pre_conv_features path (the 9x fan-out) ----------------
            for b in range(BPC):
                fx = fpool.tile([128, FPP], F32, name="fx")
                nc.scalar.dma_start(
                    out=fx[:, :], in_=feats[b].rearrange("(p a) c -> p (a c)", p=128)
                )
                for k in range(K):
                    s = K - 1 - k
                    off = s * C  # float offset of the shifted copy
                    dst = pre[b, k].rearrange("s c -> (s c)")
                    if off == 0:
                        nc.sync.dma_start(
                            out=dst.rearrange("(p f) -> p f", p=128), in_=fx[:, :]
                        )
                    else:
                        nc.sync.dma_start(
                            out=dst[0:off].unsqueeze(0), in_=zt[0:1, 0:off]
                        )
                        nc.sync.dma_start(
                            out=dst[off : off + 127 * FPP].rearrange(
                                "(p f) -> p f", p=127
                            ),
                            in_=fx[0:127, :],
                        )
                        nc.sync.dma_start(
                            out=dst[off + 127 * FPP : S * C].unsqueeze(0),
                            in_=fx[127:128, 0 : FPP - off],
                        )
    # Bacc finalize runs the legalization pipeline (wait splitting, reg alloc).
    nc.finalize()
    return nc


def _get_nc():
    global _NC
    if _NC is None:
        _NC = _build_nc()
    return _NC


def _run(in_maps, trace=False, **kwargs):
    from concourse.bass_utils import run_bass_kernel_spmd

    return run_bass_kernel_spmd(
        _get_nc(), in_maps, list(range(N_CORES)), trace=trace, **kwargs
    )


def _make_in_maps(times, features):
    times = np.ascontiguousarray(np.asarray(times), dtype=np.float32)
    features = np.ascontiguousarray(np.asarray(features), dtype=np.float32)
    return [
        {
            "times": np.ascontiguousarray(times[c * BPC : (c + 1) * BPC]),
            "features": np.ascontiguousarray(features[c * BPC : (c + 1) * BPC]),
        }
        for c in range(N_CORES)
    ]


def _assemble(results):
    delta = np.concatenate([r["delta"] for r in results], axis=0)
    pre = np.concatenate([r["pre"] for r in results], axis=0)
    return delta, pre


def kernel(times, features):
    res = _run(_make_in_maps(times, features))
    return _assemble(res.results)
